# revision 13
# baseline (speedup 1.0000x reference)
"""Trainium2 Bass kernel for a Bahdanau-attention decoder step.

Computes, for B=16, L=4096, A=512, H=512:
    dec  = concat(h, c) @ W.T + b                      # [B, A]
    e    = sum_a v[a] * tanh(feat[b,l,a] + dec[b,a])   # [B, L]
    attn = softmax(e) * mask, renormalized             # [B, L]
    ctx  = sum_l attn[b,l] * state[b,l,a]              # [B, A]

Sharding: data-parallel over batch B across 8 NeuronCores (2 rows/core).
The tiny dec projection (16x1024 @ 1024x512) is done host-side; v, dec and
mask are passed pre-transposed so the device kernel streams the two 134MB
tensors exactly once each (memory-bound target).

Device dataflow per core (2 batch rows, 8 L-groups of 512 per row):
  DMA   : feat group (128p=L, 2048f) 1MB contiguous loads
  PE    : transpose 128x128 blocks of feat -> PSUM (A on partitions)
  ACT   : tanh(psum + decT[a]) PSUM->SBUF  (bias = per-partition dec, so the
          add is free and this op is also the PSUM evacuation)
  PE    : e = vT.T @ tanhT  (v as 1-col stationary operand) -> PSUM [1, 512]
  ACT   : w = exp(e)  (no max-subtraction: |e| <~ 40 so fp32 exp is safe)
  PE    : transpose w rows -> wT columns [128, 4]
  DVE   : W_sb[:, cols] = wT * maskT  (mask fused into the PSUM evacuation)
  PE    : ctx += W_sb[:,u].T @ state tile   (state streamed in natural layout)
  finale: denom = sum(W_sb) via ones-matmul, reciprocal, scale, transpose out.
"""

import os
import sys

import numpy as np

sys.path.insert(0, "/opt/trn_rl_repo")

B, L, A, H = 16, 4096, 512, 512
NCORES = 8
BPC = B // NCORES      # batch rows per core
G = 8                  # L-groups per batch row (512 L each)
J = 4                  # 128-L subtiles per group
C = 4                  # 128-A chunks
P = 128

_CACHE = {}
LAST_RESULTS = None    # BassKernelResults of the most recent run (for test.py)
TRACE = False


def _build_nc():
    import concourse.bass as bass
    import concourse.tile as tile
    from concourse import masks, mybir

    f32 = mybir.dt.float32
    Act = mybir.ActivationFunctionType

    nc = bass.Bass()

    feat = nc.dram_tensor("feat", [BPC, L, A], f32, kind="ExternalInput")
    state = nc.dram_tensor("state", [BPC, L, A], f32, kind="ExternalInput")
    decT = nc.dram_tensor("decT", [P, C * BPC], f32, kind="ExternalInput")
    vT = nc.dram_tensor("vT", [P, C], f32, kind="ExternalInput")
    maskT = nc.dram_tensor("maskT", [P, BPC * 32], f32, kind="ExternalInput")
    ctx_out = nc.dram_tensor("ctx", [BPC, A], f32, kind="ExternalOutput")
    attn_out = nc.dram_tensor("attn", [BPC, L], f32, kind="ExternalOutput")

    # [b, g, p, (j a)] view: group g holds L rows 512g..512g+511; partition p
    # carries L row 512g + 128j + p, free = (j, a).
    featR = feat.rearrange("b (g j p) a -> b g p j a", g=G, j=J, p=P)
    stateR = state.rearrange("b (g j p) a -> b g p j a", g=G, j=J, p=P)
    attnR = attn_out.rearrange("b (u q) -> b u q", u=32)

    with tile.TileContext(nc) as tc:
        with (
            tc.tile_pool(name="consts", bufs=1) as consts,
            tc.tile_pool(name="fpool", bufs=3) as fpool,
            tc.tile_pool(name="spool", bufs=4) as spool,
            tc.tile_pool(name="tanhp", bufs=8) as tanhp,
            tc.tile_pool(name="wp", bufs=3) as wp,
            tc.tile_pool(name="wsbp", bufs=2) as wsbp,
            tc.tile_pool(name="finsb", bufs=2) as finsb,
            tc.tile_pool(name="ftps", bufs=1, space="PSUM") as ftps,
            tc.tile_pool(name="eps", bufs=1, space="PSUM") as eps,
            tc.tile_pool(name="ctxps", bufs=1, space="PSUM") as ctxps,
            tc.tile_pool(name="finps", bufs=1, space="PSUM") as finps,
        ):
            ident = consts.tile([P, P], f32)
            masks.make_identity(nc, ident[:])
            ones_col = consts.tile([P, 1], f32)
            nc.gpsimd.memset(ones_col[:], 1.0)
            ones_row = consts.tile([1, P], f32)
            nc.gpsimd.memset(ones_row[:], 1.0)

            decT_sb = consts.tile([P, C * BPC], f32)
            nc.sync.dma_start(out=decT_sb[:], in_=decT[:])
            vT_sb = consts.tile([P, C], f32)
            nc.sync.dma_start(out=vT_sb[:], in_=vT[:])
            maskT_sb = consts.tile([P, BPC * 32], f32)
            nc.sync.dma_start(out=maskT_sb[:], in_=maskT[:])

            # Single-wait observers: walrus allows only ONE semaphore wait
            # per compute instruction, so each engine "observes" the DMA'd
            # constants once up front; later ops then carry only their data
            # wait.
            sc_a = consts.tile([P, C * BPC], f32)
            nc.scalar.copy(sc_a[:], decT_sb[:])       # ACT observes decT DMA
            sc_d = consts.tile([P, 1], f32)
            nc.vector.tensor_copy(sc_d[:], maskT_sb[:, :1])  # DVE observes maskT
            sc_w = consts.tile([P, J], f32)
            sc_r = consts.tile([1, 1], f32)

            # PSUM tiles are allocated once and reused across groups and
            # batch rows: same-tile WAW/WAR hazards stay in PE program order
            # (no pool release pseudo-instructions, which would add extra
            # semaphore waits to PE matmuls — walrus allows only one).
            fts = [ftps.tile([P, J * P], f32, tag=f"ft{i}", name=f"ft{i}") for i in range(C)]
            e_row = eps.tile([P, J * P], f32, tag="erow")
            e_col = eps.tile([P, J], f32, tag="ecol")
            fin_ps = finps.tile([P, 1 + 2 * P], f32, tag="fin")
            ctx_ps = ctxps.tile([1, A], f32, tag="ctx")

            prev_mask = None
            for b in range(BPC):
                W_sb = wsbp.tile([P, 32], f32, tag="wsb")

                # Per-batch claim transposes: each carries the one wait that
                # lets PE observe the producers/readers of the tiles the
                # matmuls below touch (walrus allows one wait per compute
                # instruction). ctx-mm u==0 overwrites the scratch
                # (start=True).
                if b == 0:
                    nc.tensor.transpose(ctx_ps[:1, :P], ones_col[:], ident[:])
                    nc.tensor.transpose(ctx_ps[:1, :P], vT_sb[:, :1], ident[:])
                else:
                    # observe the previous batch finalization's readers
                    nc.tensor.transpose(ctx_ps[:1, :P], ones_col[:], ident[:])

                for g in range(G):
                    F_g = fpool.tile([P, J, A], f32, tag="fg")
                    nc.sync.dma_start(out=F_g[:], in_=featR[b, g])
                    S_g = spool.tile([P, J, A], f32, tag="sg")
                    nc.sync.dma_start(out=S_g[:], in_=stateR[b, g])

                    # feat 128x128 blocks -> PSUM, A on partitions
                    for c in range(C):
                        for j in range(J):
                            nc.tensor.transpose(
                                fts[c][:, j * P : (j + 1) * P],
                                F_g[:, j, c * P : (c + 1) * P],
                                ident[:],
                            )

                    # tanh(feat.T + dec) — bias is per-partition dec chunk
                    ths = [tanhp.tile([P, J * P], f32, tag="th", name=f"th{b}_{g}_{i}") for i in range(C)]
                    for c in range(C):
                        nc.scalar.activation(
                            ths[c][:],
                            fts[c][:],
                            Act.Tanh,
                            bias=decT_sb[:, c * BPC + b : c * BPC + b + 1],
                        )

                    # e row [1, 512] and its column transpose share one
                    # PSUM tile (same-tile PE WAW needs no semaphore).
                    for c in range(C):
                        nc.tensor.matmul(
                            e_row[:1, :],
                            vT_sb[:, c : c + 1],
                            ths[c][:],
                            start=(c == 0),
                            stop=(c == C - 1),
                        )

                    # evacuate e through ACT so the e-transposes below depend
                    # only on ACT (single-wait rule)
                    e_sb = wp.tile([1, J * P], f32, tag="esb")
                    nc.scalar.copy(e_sb[:], e_row[:1, :])

                    # e row -> column layout [128, J] (K=1 transposes)
                    for j in range(J):
                        nc.tensor.transpose(
                            e_col[:, j : j + 1],
                            e_sb[:1, j * P : (j + 1) * P],
                            ident[:1, :1],
                        )

                    # ACT observes the previous DVE mask-mul tick so exp
                    # only carries its PE wait
                    if prev_mask is not None:
                        nc.scalar.copy(sc_w[:], prev_mask)

                    # exp on the column layout (also the PSUM evacuation)
                    wraw = wp.tile([P, J], f32, tag="wraw")
                    nc.scalar.activation(wraw[:], e_col[:, :J], Act.Exp)

                    # apply mask on DVE into the per-batch weight matrix
                    nc.vector.tensor_mul(
                        W_sb[:, J * g : J * (g + 1)],
                        wraw[:],
                        maskT_sb[:, 32 * b + J * g : 32 * b + J * (g + 1)],
                    )
                    prev_mask = W_sb[:, J * g : J * (g + 1)]

                    # PE touch: observe DVE's mask tick (W_sb) so the ctx
                    # matmuls below only carry the state-DMA wait. Scratch
                    # lands in the already-consumed fts[0] region.
                    nc.tensor.transpose(
                        fts[0][:J, :P],
                        W_sb[:, J * g : J * (g + 1)],
                        ident[:],
                    )

                    # ctx accumulation over this group's 4 L-subtiles
                    for j in range(J):
                        u = J * g + j
                        nc.tensor.matmul(
                            ctx_ps[:],
                            W_sb[:, u : u + 1],
                            S_g[:, j, :],
                            start=(u == 0),
                            stop=(u == 32 - 1),
                        )

                # ---- batch-row finalization ----
                # fin_ps regions (all disjoint): col 0 = reciprocal
                # broadcast, cols 1..128 = rowsum transpose, cols 129..256 =
                # attn transpose. The claim lets PE observe the previous
                # batch's ACT reader of the attn region.
                nc.tensor.transpose(fin_ps[:1, 1 + P : 1 + 2 * P], ones_col[:], ident[:])

                rowsum = finsb.tile([P, 1], f32, tag="rowsum")
                nc.vector.tensor_reduce(
                    rowsum[:], W_sb[:], mybir.AxisListType.X, mybir.AluOpType.add
                )
                nc.tensor.transpose(fin_ps[:1, 1 : 1 + P], rowsum[:], ident[:])
                denom = finsb.tile([1, 1], f32, tag="denom")
                nc.vector.tensor_reduce(
                    denom[:], fin_ps[:1, 1 : 1 + P], mybir.AxisListType.X,
                    mybir.AluOpType.add,
                )
                recip = finsb.tile([1, 1], f32, tag="recip")
                nc.vector.reciprocal(recip[:], denom[:])

                # broadcast 1/denom to all partitions via K=1 outer product
                nc.tensor.matmul(
                    fin_ps[:, 0:1], ones_row[:], recip[:], start=True, stop=True
                )
                rb = finsb.tile([P, 1], f32, tag="rb")
                nc.vector.tensor_copy(rb[:], fin_ps[:, 0:1])

                attn_scaled = finsb.tile([P, 32], f32, tag="ascaled")
                nc.vector.tensor_scalar_mul(attn_scaled[:], W_sb[:], rb[:])

                nc.tensor.transpose(
                    fin_ps[:32, 1 + P : 1 + 2 * P], attn_scaled[:], ident[:]
                )
                nc.scalar.copy(sc_r[:], recip[:])  # ACT observes DVE recip
                attn_sb = finsb.tile([32, P], f32, tag="asb")
                nc.scalar.copy(attn_sb[:], fin_ps[:32, 1 + P : 1 + 2 * P])
                nc.sync.dma_start(out=attnR[b], in_=attn_sb[:])

                ctx_sb = finsb.tile([1, A], f32, tag="ctxsb")
                nc.scalar.mul(ctx_sb[:], ctx_ps[:], recip[:1, :1])
                nc.sync.dma_start(out=ctx_out[b : b + 1, :], in_=ctx_sb[:])

    _split_multiwaits(nc)
    return nc


def _split_multiwaits(nc):
    """Walrus in this toolchain accepts only ONE semaphore wait per compute
    instruction. Tile occasionally emits 2 (data dep + hazard dep on another
    engine). Splitting is semantics-preserving: engine streams execute in
    order, so hoisting extra waits onto same-engine NoOps immediately before
    the instruction blocks identically."""
    import concourse.mybir as mybir

    n_split = 0
    for fn in nc.m.functions:
        for blk in fn.blocks:
            insts = blk.instructions
            i = 0
            while i < len(insts):
                inst = insts[i]
                t = type(inst).__name__
                si = getattr(inst, "sync_info", None)
                eng = getattr(inst, "engine", None)
                engname = str(eng).split(".")[-1] if eng is not None else ""
                if (
                    si is not None
                    and si.on_wait
                    and len(si.on_wait) > 1
                    and engname in ("Activation", "PE", "DVE", "Pool", "SP")
                ):
                    waits = list(si.on_wait)
                    for k, w in enumerate(waits[:-1]):
                        nop = mybir.InstNoOp(name=f"{inst.name}-ws{k}", engine=eng)
                        nop.sync_info = mybir.SyncInfo(on_wait=[w], on_update=[])
                        insts.insert(i, nop)
                        i += 1
                    inst.sync_info = mybir.SyncInfo(
                        on_wait=[waits[-1]], on_update=list(si.on_update or [])
                    )
                    n_split += 1
                i += 1
    return nc


def _get_nc():
    if "nc" not in _CACHE:
        _CACHE["nc"] = _build_nc()
    return _CACHE["nc"]


def kernel(encoder_features, h, c, encoder_state, encoder_mask, v, W, b):
    global LAST_RESULTS
    from concourse.bass_utils import run_bass_kernel_spmd

    ef = np.ascontiguousarray(np.asarray(encoder_features, np.float32)).reshape(B, L, A)
    es = np.ascontiguousarray(np.asarray(encoder_state, np.float32)).reshape(B, L, A)
    h = np.asarray(h, np.float32)
    c = np.asarray(c, np.float32)
    mask = np.asarray(encoder_mask, np.float32)
    v = np.asarray(v, np.float32)
    W = np.asarray(W, np.float32)
    bb = np.asarray(b, np.float32)

    # dec = [h, c] @ W.T + b  (tiny: 16x1024 @ 1024x512)
    dec = np.concatenate([h, c], axis=1) @ W.T + bb  # [B, A]

    vT = np.ascontiguousarray(v.reshape(C, P).T)  # [p, c] -> v[128c + p]

    in_maps = []
    for k in range(NCORES):
        sl = slice(BPC * k, BPC * (k + 1))
        dec_l = dec[sl]  # [BPC, A]
        # decT[p, c*BPC + b] = dec[b, 128c + p]
        decT = np.ascontiguousarray(
            dec_l.T.reshape(C, P, BPC).transpose(1, 0, 2).reshape(P, C * BPC)
        )
        # maskT[p, 32b + u] = mask[b, 128u + p]
        maskT = np.ascontiguousarray(
            np.concatenate(
                [mask[BPC * k + i].reshape(32, P).T for i in range(BPC)], axis=1
            )
        )
        in_maps.append(
            {
                "feat": np.ascontiguousarray(ef[sl]),
                "state": np.ascontiguousarray(es[sl]),
                "decT": decT,
                "vT": vT,
                "maskT": maskT,
            }
        )

    nc = _get_nc()
    res = run_bass_kernel_spmd(
        nc,
        in_maps,
        core_ids=list(range(NCORES)),
        trace=TRACE or bool(int(os.environ.get("KERNEL_TRACE", "0"))),
    )
    LAST_RESULTS = res

    context = np.concatenate([r["ctx"] for r in res.results], axis=0)
    attn = np.concatenate([r["attn"] for r in res.results], axis=0)
    return context, attn
